# revision 1
# baseline (speedup 1.0000x reference)
"""Distributed brute-force KNN (retrieval) kernel for one TRN2 chip (8 NeuronCores).

Problem: queries [256,128] f32, candidates [500000,128] f32, identifiers [500000] i32,
k=100. Output: (values [256,100] f32 desc-sorted, ids [256,100] i32).

Strategy (v2 — group-max fold, no on-device extraction):
  - Shard candidates over N across the 8 cores (62500 each, zero-padded to
    63488 = 31 chunks x 2048).
  - Per core: bf16 matmul (Q stationary) -> PSUM f32 score chunks
    [128q, 2048c] (4 banks). Each chunk is folded by pairwise max down
    to 2048/FOLD bf16 group-maxima (member j of slot i is local col
    chunk*2048 + i + (2048/FOLD)*j). Fold1 mixes a PSUM operand with a
    ScalarE-evacuated SBUF operand (flows A/C balance ACT vs DVE). All
    slots per query-half accumulate in SBUF; one DMA out per half.
  - Host: rescore the top-C claimed groups exactly in f64, derive the
    device claim error bound, extend the selection to every group whose
    claim could still reach top-k, and take the exact top-k. Exactness
    never depends on device numerics (groups cover ALL candidates).
"""
import numpy as np
import ml_dtypes

B = 256          # queries
N = 500000       # candidates
D = 128          # dim
NCORES = 8
NSH = N // NCORES          # 62500 real candidates per core
CHUNK = 1024               # candidates per fold unit (2 PSUM banks)
NCHUNK = 62                # chunks per core
NSHP = NCHUNK * CHUNK      # 63488 padded candidates per core
FOLD = 4                   # candidates per claimed slot
SLOTS = CHUNK // FOLD      # slots per chunk
NSLOT = NCHUNK * SLOTS     # slots per (core, query)

_CACHE = {}


def build(loops=1, variant="mix", amod=15, athr=7, cbufs=6):
    """Build + compile the per-core Bass program. Returns the compiled Bacc.

    Two evacuation flows per 2048-chunk (TT with both operands in PSUM is
    an ISA violation, so fold1 always has >=1 SBUF operand):
      A: ScalarE copies all 2048 f32 PSUM -> SBUF bf16; VectorE does 4
         bf16 folds (2048->128) at 2x.
      C: ScalarE copies cols [1024:2048] only; VectorE fold1 is a mixed
         TT (PSUM f32 x SBUF bf16 -> bf16, 1x), then 3 bf16 folds.
    variant "mix": unit u is flow A iff (u % amod) < athr (default 7/12
    ~ the ACT/DVE balance point); "allact"/"allc" force one flow.
    """
    import concourse.bass as bass
    import concourse.tile as tile
    from concourse import bacc, mybir

    bf16 = mybir.dt.bfloat16
    f32 = mybir.dt.float32
    Copy = mybir.ActivationFunctionType.Copy

    nc = bacc.Bacc("TRN2", debug=False)
    qt = nc.dram_tensor("qt", [D, B], bf16, kind="ExternalInput").ap()
    ct = nc.dram_tensor("ct", [NCHUNK, D, CHUNK], bf16, kind="ExternalInput").ap()
    v8 = nc.dram_tensor("v8", [B, NSLOT], bf16, kind="ExternalOutput").ap()

    def flow_of(u):
        if variant == "allact":
            return "A"
        if variant == "allc" or variant == "3way":
            return "C"
        return "A" if (u % amod) < athr else "C"

    with tile.TileContext(nc) as tc:
        with (
            tc.tile_pool(name="qpool", bufs=1) as qpool,
            tc.tile_pool(name="cpool", bufs=cbufs) as cpool,
            tc.tile_pool(name="psum", bufs=4, space="PSUM") as pp,
            tc.tile_pool(name="evac", bufs=4) as epool,
            tc.tile_pool(name="fold", bufs=4) as fpool,
            tc.tile_pool(name="acc", bufs=1) as accp,
        ):
            qtile = qpool.tile([D, B], bf16)
            nc.sync.dma_start(qtile[:], qt[:])
            vacc = [
                accp.tile([128, NSLOT], bf16, tag=f"vacc{h}", name=f"vacc{h}")
                for h in range(2)
            ]

            def body(_iv=None):
                u = 0
                for cc in range(NCHUNK // 2):
                    ctile = cpool.tile([D, 2 * CHUNK], bf16, tag="ct", name="ctile")
                    for s2 in range(2):
                        nc.sync.dma_start(
                            ctile[:, bass.ds(s2 * CHUNK, CHUNK)],
                            ct[2 * cc + s2, :, :],
                        )
                    for sub in range(2):
                      c = 2 * cc + sub
                      for h in range(2):
                        ps = pp.tile([128, CHUNK], f32, name="ps")
                        for j in range(2):
                            nc.tensor.matmul(
                                ps[:, bass.ds(j * 512, 512)],
                                lhsT=qtile[:, bass.ds(h * 128, 128)],
                                rhs=ctile[:, bass.ds(sub * CHUNK + j * 512, 512)],
                                start=True,
                                stop=True,
                            )
                        if variant.startswith("nothing"):
                            u += 1
                            continue
                        if variant == "folds0":
                            sc = epool.tile([128, CHUNK], bf16, tag="sc", name="sc")
                            nc.scalar.activation(sc[:], ps[:], Copy)
                            u += 1
                            continue
                        f1 = fpool.tile([128, CHUNK // 2], bf16, tag="f1", name="f1")
                        if variant == "3way" and (u % amod) < athr:
                            sc = epool.tile([128, CHUNK], bf16, tag="sc", name="sc")
                            nc.scalar.activation(sc[:], ps[:], Copy)
                            nc.gpsimd.tensor_max(
                                f1[:],
                                sc[:, bass.ds(0, 1024)],
                                sc[:, bass.ds(1024, 1024)],
                            )
                            w = 1024
                            cur = f1
                            while w // 2 > SLOTS:
                                w //= 2
                                nxt = fpool.tile(
                                    [128, w], bf16, tag=f"f{w}", name=f"f{w}"
                                )
                                nc.gpsimd.tensor_max(
                                    nxt[:],
                                    cur[:, bass.ds(0, w)],
                                    cur[:, bass.ds(w, w)],
                                )
                                cur = nxt
                            nc.gpsimd.tensor_max(
                                vacc[h][:, bass.ds(c * SLOTS, SLOTS)],
                                cur[:, bass.ds(0, SLOTS)],
                                cur[:, bass.ds(SLOTS, SLOTS)],
                            )
                            u += 1
                            continue
                        if flow_of(u) == "A":
                            sc = epool.tile([128, CHUNK], bf16, tag="sc", name="sc")
                            nc.scalar.activation(sc[:], ps[:], Copy)
                            nc.vector.tensor_max(
                                f1[:],
                                sc[:, bass.ds(0, CHUNK // 2)],
                                sc[:, bass.ds(CHUNK // 2, CHUNK // 2)],
                            )
                        else:
                            sc = epool.tile(
                                [128, CHUNK // 2], bf16, tag="sc2", name="sc2"
                            )
                            nc.scalar.activation(
                                sc[:], ps[:, bass.ds(CHUNK // 2, CHUNK // 2)], Copy
                            )
                            nc.vector.tensor_max(
                                f1[:], ps[:, bass.ds(0, CHUNK // 2)], sc[:]
                            )
                        # remaining bf16 folds down to SLOTS wide
                        w = CHUNK // 2
                        cur = f1
                        while w // 2 > SLOTS:
                            w //= 2
                            nxt = fpool.tile([128, w], bf16, tag=f"f{w}", name=f"f{w}")
                            nc.vector.tensor_max(
                                nxt[:], cur[:, bass.ds(0, w)], cur[:, bass.ds(w, w)]
                            )
                            cur = nxt
                        nc.vector.tensor_max(
                            vacc[h][:, bass.ds(c * SLOTS, SLOTS)],
                            cur[:, bass.ds(0, SLOTS)],
                            cur[:, bass.ds(SLOTS, SLOTS)],
                        )
                        u += 1

            if loops == 1:
                body()
            else:
                with tc.For_i(0, loops, 1) as iv:
                    body(iv)

            if variant != "folds0" and not variant.startswith("nothing"):
                for h in range(2):
                    nc.sync.dma_start(v8[bass.ds(h * 128, 128), :], vacc[h][:])
    nc.compile()
    return nc


def _get_nc():
    if "nc" not in _CACHE:
        _CACHE["nc"] = build()
    return _CACHE["nc"]


def make_in_maps(queries, candidates):
    qt = np.ascontiguousarray(queries.T).astype(ml_dtypes.bfloat16)
    cb = candidates.astype(ml_dtypes.bfloat16)
    in_maps = []
    for c in range(NCORES):
        flat = np.zeros((D, NSHP), dtype=ml_dtypes.bfloat16)
        flat[:, :NSH] = cb[c * NSH : (c + 1) * NSH].T
        ct = np.ascontiguousarray(
            flat.reshape(D, NCHUNK, CHUNK).transpose(1, 0, 2)
        )
        in_maps.append({"qt": qt, "ct": ct})
    return in_maps


def _device_claims(queries, candidates):
    """Run the 8-core SPMD kernel; return claims [NCORES, B, NSLOT] f32."""
    from concourse.bass_utils import run_bass_kernel_spmd

    nc = _get_nc()
    in_maps = make_in_maps(queries, candidates)
    res = None
    for attempt in range(3):
        try:
            res = run_bass_kernel_spmd(nc, in_maps, core_ids=list(range(NCORES))).results
            break
        except Exception:
            if attempt == 2:
                raise
            import time as _time

            _time.sleep(2.0)
    assert res is not None
    return np.stack([r["v8"] for r in res]).astype(np.float32)


def kernel(queries, candidates, identifiers, k):
    queries = np.asarray(queries, dtype=np.float32)
    candidates = np.asarray(candidates, dtype=np.float32)
    identifiers = np.asarray(identifiers)
    kk = int(k)

    v8 = _device_claims(queries, candidates)            # [8, B, NSLOT]

    # flatten claims to [B, NCORES*NSLOT]; group g = (core, slotcol)
    vals = v8.transpose(1, 0, 2).reshape(B, NCORES * NSLOT)

    q64 = queries.astype(np.float64)
    sigma = np.linalg.norm(queries, axis=1)

    # group id -> member global candidate indices [..., FOLD] (or <0 invalid)
    def members_of(g):
        core, sl = g // NSLOT, g % NSLOT
        c, i = sl // SLOTS, sl % SLOTS
        L = (c * CHUNK + i)[..., None] + SLOTS * np.arange(FOLD)
        valid = L < NSH
        gl = L + (core * NSH)[..., None]
        return np.where(valid, gl, -1)

    def rescore(mem, qidx):
        """mem [Q, M, FOLD] global ids (-1 invalid) -> exact f64 scores."""
        Q = mem.shape[0]
        out = np.empty(mem.shape, np.float64)
        step = 64
        for s in range(0, Q, step):
            e = min(s + step, Q)
            blk = mem[s:e]
            safe = np.where(blk >= 0, blk, 0)
            sv = np.einsum(
                "qmfd,qd->qmf",
                candidates[safe].astype(np.float64),
                q64[qidx[s:e]],
            )
            out[s:e] = np.where(blk >= 0, sv, -np.inf)
        return out

    # --- preselect top-C groups per query, rescore exactly ---
    C = max(2 * kk, kk + 64)
    part = np.argpartition(-vals, C, axis=1)[:, :C]
    vsel = np.take_along_axis(vals, part, 1)
    mem = members_of(part)                              # [B, C, FOLD]
    allq = np.arange(B)
    se = rescore(mem, allq)                             # [B, C, FOLD]
    gmax = se.max(2)
    finite = np.isfinite(gmax)
    delta = np.where(finite, np.abs(vsel - gmax), 0.0).max(1)
    margin = 4.0 * delta + 1e-3 * sigma

    flat = se.reshape(B, -1)
    vk = -np.partition(-flat, kk - 1, axis=1)[:, kk - 1]
    thr = vk - margin

    pool_v = [flat[q] for q in range(B)]
    pool_g = [mem[q].reshape(-1) for q in range(B)]

    # any group above thr that wasn't rescored yet
    selmask = np.zeros(vals.shape, dtype=bool)
    np.put_along_axis(selmask, part, True, 1)
    need = (vals >= thr[:, None]) & ~selmask
    for q in np.nonzero(need.any(1))[0]:
        g = np.nonzero(need[q])[0]
        m = members_of(g)[None]                          # [1, M, FOLD]
        sv = rescore(m, np.array([q]))[0]
        pool_v[q] = np.concatenate([pool_v[q], sv.reshape(-1)])
        pool_g[q] = np.concatenate([pool_g[q], m[0].reshape(-1)])

    # --- final exact top-k per query (dedupe, desc value, index tiebreak) --
    out_v = np.empty((B, kk), np.float32)
    out_g = np.empty((B, kk), np.int64)
    for q in range(B):
        keep = pool_g[q] >= 0
        g, first = np.unique(pool_g[q][keep], return_index=True)
        v32 = pool_v[q][keep][first].astype(np.float32)
        assert v32.size >= kk
        order = np.lexsort((g, -v32))[:kk]
        out_v[q] = v32[order]
        out_g[q] = g[order]

    top_ids = identifiers[out_g]
    return out_v, top_ids



# revision 2
# speedup vs baseline: 1.1503x; 1.1503x over previous
"""Distributed brute-force KNN (retrieval) kernel for one TRN2 chip (8 NeuronCores).

Problem: queries [256,128] f32, candidates [500000,128] f32, identifiers [500000] i32,
k=100. Output: (values [256,100] f32 desc-sorted, ids [256,100] i32).

Strategy (v3 — FOLD=2 pair-max claims, drain-balanced flows):
  - Shard candidates over N across the 8 cores (62500 each, zero-padded to
    63488 = 31 chunks x 2048).
  - Per core, per chunk (2048 cands) and query-half (128 queries): 4 bf16
    matmuls (Q stationary) -> one PSUM f32 tile [128, 2048] (4 banks).
  - Drain+fold to pair-max claims: slot j of a chunk claims
    max(s[j], s[j+1024]) in bf16 (members {base+j, base+j+1024}).
    Two pipelined flows balance ScalarE vs VectorE:
      A: ACT copies all 2048 f32 PSUM -> SBUF bf16 (1x); DVE folds with a
         bf16 tensor-max at 2x.
      C: ACT copies cols [1024:2048] only; DVE tensor-max mixes the PSUM
         f32 left half with the evacuated bf16 right half (1x).
      V: DVE reduce_max over a [128,1024,2]-strided PSUM view (1x,
         drains+folds in one op, zero ACT).
    Default pattern 8C:1A per the engine-balance LP (ACT ~ DVE ~ 70us).
  - Host: rescore the top-C claimed groups exactly in f64, derive the
    device claim error bound, extend the selection to every group whose
    claim could still reach top-k, and take the exact top-k. Exactness
    never depends on device numerics (groups cover ALL candidates).
"""
import numpy as np
import ml_dtypes

B = 256          # queries
N = 500000       # candidates
D = 128          # dim
NCORES = 8
NSH = N // NCORES          # 62500 real candidates per core
CHUNK = 2048               # candidates per chunk (4 PSUM banks f32)
NCHUNK = 31                # chunks per core
NSHP = NCHUNK * CHUNK      # 63488 padded candidates per core
FOLD = 2                   # candidates per claimed slot
SLOTS = CHUNK // FOLD      # 1024 slots per chunk
NSLOT = NCHUNK * SLOTS     # 31744 slots per (core, query)
PAIR_STRIDE = CHUNK // 2   # member j partner offset within chunk

_CACHE = {}


def build(loops=1, pattern="CCCCCCCCA", cbufs=4, ebufs=4):
    """Build + compile the per-core Bass program. Returns the compiled Bacc.

    pattern: flow letter per unit (cycled), over:
      A: ACT evacuates all 2048 (1x); DVE bf16 pair-max at 2x.
      C: ACT evacuates cols [1024:2048]; DVE pair-max of PSUM f32 x SBUF
         bf16 at 1x (drains the left half while folding).
      V: DVE reduce_max over strided [128,1024,2] PSUM view (1x, no ACT).
    All flows produce the same slot mapping: slot j = max(s_j, s_{j+1024}).
    """
    import concourse.bass as bass
    import concourse.tile as tile
    from concourse import bacc, mybir

    bf16 = mybir.dt.bfloat16
    f32 = mybir.dt.float32
    Copy = mybir.ActivationFunctionType.Copy

    nc = bacc.Bacc("TRN2", debug=False)
    qt = nc.dram_tensor("qt", [D, B], bf16, kind="ExternalInput").ap()
    ct = nc.dram_tensor("ct", [NCHUNK, D, CHUNK], bf16, kind="ExternalInput").ap()
    v8 = nc.dram_tensor("v8", [B, NSLOT], bf16, kind="ExternalOutput").ap()

    with tile.TileContext(nc) as tc:
        with (
            tc.tile_pool(name="qpool", bufs=1) as qpool,
            tc.tile_pool(name="cpool", bufs=cbufs) as cpool,
            tc.tile_pool(name="psum", bufs=2, space="PSUM") as pp,
            tc.tile_pool(name="evac", bufs=ebufs) as epool,
            tc.tile_pool(name="acc", bufs=1) as accp,
        ):
            qtile = qpool.tile([D, B], bf16)
            nc.sync.dma_start(qtile[:], qt[:])
            vacc = [
                accp.tile([128, NSLOT], bf16, tag=f"vacc{h}", name=f"vacc{h}")
                for h in range(2)
            ]

            def body(_iv=None):
                u = 0
                for cc in range(NCHUNK):
                    ctile = cpool.tile([D, CHUNK], bf16, tag="ct", name="ctile")
                    nc.sync.dma_start(ctile[:], ct[cc, :, :])
                    for h in range(2):
                        flow = pattern[u % len(pattern)]
                        dst = vacc[h][:, bass.ds(cc * SLOTS, SLOTS)]
                        ps = pp.tile([128, CHUNK], f32, name="ps")
                        for j in range(4):
                            nc.tensor.matmul(
                                ps[:, bass.ds(j * 512, 512)],
                                lhsT=qtile[:, bass.ds(h * 128, 128)],
                                rhs=ctile[:, bass.ds(j * 512, 512)],
                                start=True,
                                stop=True,
                            )
                        if flow == "A":
                            sc = epool.tile([128, CHUNK], bf16, tag="scA", name="scA")
                            nc.scalar.activation(sc[:], ps[:], Copy)
                            nc.vector.tensor_max(
                                dst,
                                sc[:, bass.ds(0, SLOTS)],
                                sc[:, bass.ds(SLOTS, SLOTS)],
                            )
                        elif flow == "C":
                            sc = epool.tile([128, SLOTS], bf16, tag="scC", name="scC")
                            nc.scalar.activation(
                                sc[:], ps[:, bass.ds(SLOTS, SLOTS)], Copy
                            )
                            nc.vector.tensor_max(
                                dst, ps[:, bass.ds(0, SLOTS)], sc[:]
                            )
                        else:  # "V"
                            pv = ps[:].rearrange("p (m j) -> p j m", m=2)
                            nc.vector.tensor_reduce(
                                dst, pv, axis=mybir.AxisListType.X,
                                op=mybir.AluOpType.max,
                            )
                        u += 1

            if loops == 1:
                body()
            else:
                with tc.For_i(0, loops, 1) as iv:
                    body(iv)

            for h in range(2):
                nc.sync.dma_start(v8[bass.ds(h * 128, 128), :], vacc[h][:])
    nc.compile()
    return nc


def _get_nc():
    if "nc" not in _CACHE:
        _CACHE["nc"] = build()
    return _CACHE["nc"]


def make_in_maps(queries, candidates):
    qt = np.ascontiguousarray(queries.T).astype(ml_dtypes.bfloat16)
    cb = candidates.astype(ml_dtypes.bfloat16)
    in_maps = []
    for c in range(NCORES):
        flat = np.zeros((D, NSHP), dtype=ml_dtypes.bfloat16)
        flat[:, :NSH] = cb[c * NSH : (c + 1) * NSH].T
        ct = np.ascontiguousarray(
            flat.reshape(D, NCHUNK, CHUNK).transpose(1, 0, 2)
        )
        in_maps.append({"qt": qt, "ct": ct})
    return in_maps


def _device_claims(queries, candidates):
    """Run the 8-core SPMD kernel; return claims [NCORES, B, NSLOT] f32."""
    from concourse.bass_utils import run_bass_kernel_spmd

    nc = _get_nc()
    in_maps = make_in_maps(queries, candidates)
    res = None
    for attempt in range(3):
        try:
            res = run_bass_kernel_spmd(nc, in_maps, core_ids=list(range(NCORES))).results
            break
        except Exception:
            if attempt == 2:
                raise
            import time as _time

            _time.sleep(2.0)
    assert res is not None
    return np.stack([r["v8"] for r in res]).astype(np.float32)


def kernel(queries, candidates, identifiers, k):
    queries = np.asarray(queries, dtype=np.float32)
    candidates = np.asarray(candidates, dtype=np.float32)
    identifiers = np.asarray(identifiers)
    kk = int(k)

    v8 = _device_claims(queries, candidates)            # [8, B, NSLOT]

    # flatten claims to [B, NCORES*NSLOT]; group g = (core, slotcol)
    vals = v8.transpose(1, 0, 2).reshape(B, NCORES * NSLOT)

    q64 = queries.astype(np.float64)
    sigma = np.linalg.norm(queries, axis=1)

    # group id -> member global candidate indices [..., FOLD] (or <0 invalid)
    def members_of(g):
        core, sl = g // NSLOT, g % NSLOT
        c, j = sl // SLOTS, sl % SLOTS
        L = (c * CHUNK + j)[..., None] + PAIR_STRIDE * np.arange(FOLD)
        valid = L < NSH
        gl = L + (core * NSH)[..., None]
        return np.where(valid, gl, -1)

    def rescore(mem, qidx):
        """mem [Q, M, FOLD] global ids (-1 invalid) -> exact f64 scores."""
        Q = mem.shape[0]
        out = np.empty(mem.shape, np.float64)
        step = 64
        for s in range(0, Q, step):
            e = min(s + step, Q)
            blk = mem[s:e]
            safe = np.where(blk >= 0, blk, 0)
            sv = np.einsum(
                "qmfd,qd->qmf",
                candidates[safe].astype(np.float64),
                q64[qidx[s:e]],
            )
            out[s:e] = np.where(blk >= 0, sv, -np.inf)
        return out

    # --- preselect top-C groups per query, rescore exactly ---
    C = max(2 * kk, kk + 64)
    part = np.argpartition(-vals, C, axis=1)[:, :C]
    vsel = np.take_along_axis(vals, part, 1)
    mem = members_of(part)                              # [B, C, FOLD]
    allq = np.arange(B)
    se = rescore(mem, allq)                             # [B, C, FOLD]
    gmax = se.max(2)
    finite = np.isfinite(gmax)
    delta = np.where(finite, np.abs(vsel - gmax), 0.0).max(1)
    margin = 4.0 * delta + 1e-3 * sigma

    flat = se.reshape(B, -1)
    vk = -np.partition(-flat, kk - 1, axis=1)[:, kk - 1]
    thr = vk - margin

    pool_v = [flat[q] for q in range(B)]
    pool_g = [mem[q].reshape(-1) for q in range(B)]

    # any group above thr that wasn't rescored yet
    selmask = np.zeros(vals.shape, dtype=bool)
    np.put_along_axis(selmask, part, True, 1)
    need = (vals >= thr[:, None]) & ~selmask
    for q in np.nonzero(need.any(1))[0]:
        g = np.nonzero(need[q])[0]
        m = members_of(g)[None]                          # [1, M, FOLD]
        sv = rescore(m, np.array([q]))[0]
        pool_v[q] = np.concatenate([pool_v[q], sv.reshape(-1)])
        pool_g[q] = np.concatenate([pool_g[q], m[0].reshape(-1)])

    # --- final exact top-k per query (dedupe, desc value, index tiebreak) --
    out_v = np.empty((B, kk), np.float32)
    out_g = np.empty((B, kk), np.int64)
    for q in range(B):
        keep = pool_g[q] >= 0
        g, first = np.unique(pool_g[q][keep], return_index=True)
        v32 = pool_v[q][keep][first].astype(np.float32)
        assert v32.size >= kk
        order = np.lexsort((g, -v32))[:kk]
        out_v[q] = v32[order]
        out_g[q] = g[order]

    top_ids = identifiers[out_g]
    return out_v, top_ids


# revision 12
# speedup vs baseline: 1.5570x; 1.3536x over previous
"""Distributed brute-force KNN (retrieval) kernel for one TRN2 chip (8 NeuronCores).

Problem: queries [256,128] f32, candidates [500000,128] f32, identifiers [500000] i32,
k=100. Output: (values [256,100] f32 desc-sorted, ids [256,100] i32).

Strategy (v3 — FOLD=2 pair-max claims, drain-balanced flows):
  - Shard candidates over N across the 8 cores (62500 each, zero-padded to
    63488 = 31 chunks x 2048).
  - Per core, per chunk (2048 cands) and query-half (128 queries): 4 bf16
    matmuls (Q stationary) -> one PSUM f32 tile [128, 2048] (4 banks).
  - Drain+fold to pair-max claims: slot j of a chunk claims
    max(s[j], s[j+1024]) in bf16 (members {base+j, base+j+1024}).
    Two pipelined flows balance ScalarE vs VectorE:
      A: ACT copies all 2048 f32 PSUM -> SBUF bf16 (1x); DVE folds with a
         bf16 tensor-max at 2x.
      C: ACT copies cols [1024:2048] only; DVE tensor-max mixes the PSUM
         f32 left half with the evacuated bf16 right half (1x).
      V: DVE reduce_max over a [128,1024,2]-strided PSUM view (1x,
         drains+folds in one op, zero ACT).
    Default pattern 8C:1A per the engine-balance LP (ACT ~ DVE ~ 70us).
  - Host: rescore the top-C claimed groups exactly in f64, derive the
    device claim error bound, extend the selection to every group whose
    claim could still reach top-k, and take the exact top-k. Exactness
    never depends on device numerics (groups cover ALL candidates).
"""
import numpy as np
import ml_dtypes

B = 256          # queries
N = 500000       # candidates
D = 128          # dim
NCORES = 8
NSH = N // NCORES          # 62500 real candidates per core
CHUNK = 2048               # candidates per chunk (4 PSUM banks f32)
NCHUNK = 31                # chunks per core
NSHP = NCHUNK * CHUNK      # 63488 padded candidates per core
FOLD = 2                   # candidates per claimed slot
SLOTS = CHUNK // FOLD      # 1024 slots per chunk
NSLOT = NCHUNK * SLOTS     # 31744 slots per (core, query)
PAIR_STRIDE = CHUNK // 2   # member j partner offset within chunk

_CACHE = {}


def build(loops=1, pattern="C", cbufs=4, ebufs=4, plbufs=2, prbufs=2):
    """Build + compile the per-core Bass program. Returns the compiled Bacc.

    Split-tile design: each 2048-chunk unit uses TWO 2-bank PSUM tiles —
    PR (cols 1024:2048, matmul'd first) and PL (cols 0:1024). Each PSUM
    tile has exactly ONE reader so tile recycling never waits on the
    chained ACT->DVE pair:
      C: ACT evacuates PR -> sc (overlapping PL's matmuls); DVE pair-max
         of PL (PSUM f32) x sc (SBUF bf16) at 1x.
      A: ACT evacuates PR and PL (two ops); DVE bf16 pair-max at 2x.
      V: DVE reduce_max over strided [128,1024,2] view spanning... not
         supported in split layout; kept only as diag on PL pairs.
      N: no drain (diag).
    All flows produce slot j = max(s_j, s_{j+1024}).
    """
    import concourse.bass as bass
    import concourse.tile as tile
    from concourse import bacc, mybir

    bf16 = mybir.dt.bfloat16
    f32 = mybir.dt.float32
    Copy = mybir.ActivationFunctionType.Copy

    nc = bacc.Bacc("TRN2", debug=False)
    qt = nc.dram_tensor("qt", [D, B], bf16, kind="ExternalInput").ap()
    ct = nc.dram_tensor("ct", [NCHUNK, D, CHUNK], bf16, kind="ExternalInput").ap()
    v8 = nc.dram_tensor("v8", [B, NSLOT], bf16, kind="ExternalOutput").ap()

    with tile.TileContext(nc) as tc:
        with (
            tc.tile_pool(name="qpool", bufs=1) as qpool,
            tc.tile_pool(name="cpool", bufs=cbufs) as cpool,
            tc.tile_pool(name="ppl", bufs=plbufs, space="PSUM") as ppl,
            tc.tile_pool(name="ppr", bufs=prbufs, space="PSUM") as ppr,
            tc.tile_pool(name="evac", bufs=ebufs) as epool,
            tc.tile_pool(name="acc", bufs=1) as accp,
        ):
            qtile = qpool.tile([D, B], bf16)
            nc.sync.dma_start(qtile[:], qt[:])
            vacc = [
                accp.tile([128, NSLOT], bf16, tag=f"vacc{h}", name=f"vacc{h}")
                for h in range(2)
            ]
            dummy = None
            if "D" in pattern:
                dummy = accp.tile([128, SLOTS], bf16, tag="dum", name="dum")
                nc.scalar.memzero(dummy[:])

            def body(_iv=None):
                u = 0
                pending = []

                def flush():
                    while pending:
                        pending.pop(0)()

                for cc in range(NCHUNK):
                    ctile = cpool.tile([D, CHUNK], bf16, tag="ct", name="ctile")
                    nc.sync.dma_start(ctile[:], ct[cc, :, :])
                    for h in range(2):
                        flow = pattern[u % len(pattern)]
                        dst = vacc[h][:, bass.ds(cc * SLOTS, SLOTS)]
                        lhsT = qtile[:, bass.ds(h * 128, 128)]
                        pr = ppr.tile([128, SLOTS], f32, tag="pr", name="pr")
                        pl = ppl.tile([128, SLOTS], f32, tag="pl", name="pl")
                        if flow != "N":
                            for j in range(2):
                                nc.tensor.matmul(
                                    pr[:, bass.ds(j * 512, 512)],
                                    lhsT=lhsT,
                                    rhs=ctile[:, bass.ds(SLOTS + j * 512, 512)],
                                    start=True,
                                    stop=True,
                                )
                        for j in range(2):
                            nc.tensor.matmul(
                                pl[:, bass.ds(j * 512, 512)],
                                lhsT=lhsT,
                                rhs=ctile[:, bass.ds(j * 512, 512)],
                                start=True,
                                stop=True,
                            )
                        if flow == "A":
                            # lazy emission: interleave this unit's ACT burst
                            # between neighbouring C-units' evacs so DVE's sc
                            # feed is never stalled behind a 2-op ACT run.
                            def emit_a(pr=pr, pl=pl, dst=dst):
                                sc = epool.tile(
                                    [128, CHUNK], bf16, tag="scA", name="scA"
                                )
                                nc.scalar.activation(
                                    sc[:, bass.ds(SLOTS, SLOTS)], pr[:], Copy
                                )
                                nc.scalar.activation(
                                    sc[:, bass.ds(0, SLOTS)], pl[:], Copy
                                )
                                nc.vector.tensor_max(
                                    dst,
                                    sc[:, bass.ds(0, SLOTS)],
                                    sc[:, bass.ds(SLOTS, SLOTS)],
                                )

                            pending.append(emit_a)
                        elif flow == "G":
                            # ACT evacuates both halves (lazily interleaved);
                            # GPSIMD does the pair-max — DVE untouched.
                            def emit_g(pr=pr, pl=pl, dst=dst):
                                sc = epool.tile(
                                    [128, CHUNK], f32, tag="scG", name="scG"
                                )
                                nc.scalar.activation(
                                    sc[:, bass.ds(SLOTS, SLOTS)], pr[:], Copy
                                )
                                nc.scalar.activation(
                                    sc[:, bass.ds(0, SLOTS)], pl[:], Copy
                                )
                                nc.gpsimd.tensor_max(
                                    dst,
                                    sc[:, bass.ds(0, SLOTS)],
                                    sc[:, bass.ds(SLOTS, SLOTS)],
                                )

                            pending.append(emit_g)
                        elif flow == "C":
                            sc = epool.tile([128, SLOTS], bf16, tag="scC", name="scC")
                            nc.scalar.activation(sc[:], pr[:], Copy)
                            nc.vector.tensor_max(dst, pl[:], sc[:])
                            flush()
                        elif flow == "D":  # diag: DVE mixed vs dummy (no ACT dep)
                            nc.vector.tensor_max(dst, pl[:], dummy[:])
                            # tiny ACT consume so pr isn't written-never-read
                            scm = epool.tile([128, 16], bf16, tag="scm", name="scm")
                            nc.scalar.activation(scm[:], pr[:, bass.ds(0, 16)], Copy)
                        elif flow == "S":  # diag: ACT evac only, tiny DVE
                            sc = epool.tile([128, SLOTS], bf16, tag="scC", name="scC")
                            sc2 = epool.tile([128, SLOTS], bf16, tag="sc2", name="sc2")
                            nc.scalar.activation(sc[:], pr[:], Copy)
                            nc.scalar.activation(sc2[:], pl[:], Copy)
                            nc.vector.tensor_max(
                                dst[:, bass.ds(0, 16)],
                                sc[:, bass.ds(0, 16)],
                                sc2[:, bass.ds(0, 16)],
                            )
                        u += 1
                flush()

            if loops == 1:
                body()
            else:
                with tc.For_i(0, loops, 1) as iv:
                    body(iv)

            for h in range(2):
                nc.sync.dma_start(v8[bass.ds(h * 128, 128), :], vacc[h][:])
    nc.compile()
    return nc


def _get_nc():
    if "nc" not in _CACHE:
        _CACHE["nc"] = build()
    return _CACHE["nc"]


def make_in_maps(queries, candidates):
    qt = np.ascontiguousarray(queries.T).astype(ml_dtypes.bfloat16)
    cb = candidates.astype(ml_dtypes.bfloat16)
    in_maps = []
    for c in range(NCORES):
        flat = np.zeros((D, NSHP), dtype=ml_dtypes.bfloat16)
        flat[:, :NSH] = cb[c * NSH : (c + 1) * NSH].T
        ct = np.ascontiguousarray(
            flat.reshape(D, NCHUNK, CHUNK).transpose(1, 0, 2)
        )
        in_maps.append({"qt": qt, "ct": ct})
    return in_maps


def _device_claims(queries, candidates):
    """Run the 8-core SPMD kernel; return claims [NCORES, B, NSLOT] f32."""
    from concourse.bass_utils import run_bass_kernel_spmd

    nc = _get_nc()
    in_maps = make_in_maps(queries, candidates)
    res = None
    for attempt in range(3):
        try:
            res = run_bass_kernel_spmd(nc, in_maps, core_ids=list(range(NCORES))).results
            break
        except Exception:
            if attempt == 2:
                raise
            import time as _time

            _time.sleep(2.0)
    assert res is not None
    return np.stack([r["v8"] for r in res]).astype(np.float32)


def kernel(queries, candidates, identifiers, k):
    queries = np.asarray(queries, dtype=np.float32)
    candidates = np.asarray(candidates, dtype=np.float32)
    identifiers = np.asarray(identifiers)
    kk = int(k)

    v8 = _device_claims(queries, candidates)            # [8, B, NSLOT]

    # flatten claims to [B, NCORES*NSLOT]; group g = (core, slotcol)
    vals = v8.transpose(1, 0, 2).reshape(B, NCORES * NSLOT)

    q64 = queries.astype(np.float64)
    sigma = np.linalg.norm(queries, axis=1)

    # group id -> member global candidate indices [..., FOLD] (or <0 invalid)
    def members_of(g):
        core, sl = g // NSLOT, g % NSLOT
        c, j = sl // SLOTS, sl % SLOTS
        L = (c * CHUNK + j)[..., None] + PAIR_STRIDE * np.arange(FOLD)
        valid = L < NSH
        gl = L + (core * NSH)[..., None]
        return np.where(valid, gl, -1)

    def rescore(mem, qidx):
        """mem [Q, M, FOLD] global ids (-1 invalid) -> exact f64 scores."""
        Q = mem.shape[0]
        out = np.empty(mem.shape, np.float64)
        step = 64
        for s in range(0, Q, step):
            e = min(s + step, Q)
            blk = mem[s:e]
            safe = np.where(blk >= 0, blk, 0)
            sv = np.einsum(
                "qmfd,qd->qmf",
                candidates[safe].astype(np.float64),
                q64[qidx[s:e]],
            )
            out[s:e] = np.where(blk >= 0, sv, -np.inf)
        return out

    # --- preselect top-C groups per query, rescore exactly ---
    C = max(2 * kk, kk + 64)
    part = np.argpartition(-vals, C, axis=1)[:, :C]
    vsel = np.take_along_axis(vals, part, 1)
    mem = members_of(part)                              # [B, C, FOLD]
    allq = np.arange(B)
    se = rescore(mem, allq)                             # [B, C, FOLD]
    gmax = se.max(2)
    finite = np.isfinite(gmax)
    delta = np.where(finite, np.abs(vsel - gmax), 0.0).max(1)
    margin = 4.0 * delta + 1e-3 * sigma

    flat = se.reshape(B, -1)
    vk = -np.partition(-flat, kk - 1, axis=1)[:, kk - 1]
    thr = vk - margin

    pool_v = [flat[q] for q in range(B)]
    pool_g = [mem[q].reshape(-1) for q in range(B)]

    # any group above thr that wasn't rescored yet
    selmask = np.zeros(vals.shape, dtype=bool)
    np.put_along_axis(selmask, part, True, 1)
    need = (vals >= thr[:, None]) & ~selmask
    for q in np.nonzero(need.any(1))[0]:
        g = np.nonzero(need[q])[0]
        m = members_of(g)[None]                          # [1, M, FOLD]
        sv = rescore(m, np.array([q]))[0]
        pool_v[q] = np.concatenate([pool_v[q], sv.reshape(-1)])
        pool_g[q] = np.concatenate([pool_g[q], m[0].reshape(-1)])

    # --- final exact top-k per query (dedupe, desc value, index tiebreak) --
    out_v = np.empty((B, kk), np.float32)
    out_g = np.empty((B, kk), np.int64)
    for q in range(B):
        keep = pool_g[q] >= 0
        g, first = np.unique(pool_g[q][keep], return_index=True)
        v32 = pool_v[q][keep][first].astype(np.float32)
        assert v32.size >= kk
        order = np.lexsort((g, -v32))[:kk]
        out_v[q] = v32[order]
        out_g[q] = g[order]

    top_ids = identifiers[out_g]
    return out_v, top_ids


# revision 13
# speedup vs baseline: 1.6583x; 1.0651x over previous
"""Distributed brute-force KNN (retrieval) kernel for one TRN2 chip (8 NeuronCores).

Problem: queries [256,128] f32, candidates [500000,128] f32, identifiers [500000] i32,
k=100. Output: (values [256,100] f32 desc-sorted, ids [256,100] i32).

Strategy (v3 — FOLD=2 pair-max claims, drain-balanced flows):
  - Shard candidates over N across the 8 cores (62500 each, zero-padded to
    63488 = 31 chunks x 2048).
  - Per core, per chunk (2048 cands) and query-half (128 queries): 4 bf16
    matmuls (Q stationary) -> one PSUM f32 tile [128, 2048] (4 banks).
  - Drain+fold to pair-max claims: slot j of a chunk claims
    max(s[j], s[j+1024]) in bf16 (members {base+j, base+j+1024}).
    Two pipelined flows balance ScalarE vs VectorE:
      A: ACT copies all 2048 f32 PSUM -> SBUF bf16 (1x); DVE folds with a
         bf16 tensor-max at 2x.
      C: ACT copies cols [1024:2048] only; DVE tensor-max mixes the PSUM
         f32 left half with the evacuated bf16 right half (1x).
      V: DVE reduce_max over a [128,1024,2]-strided PSUM view (1x,
         drains+folds in one op, zero ACT).
    Default pattern 8C:1A per the engine-balance LP (ACT ~ DVE ~ 70us).
  - Host: rescore the top-C claimed groups exactly in f64, derive the
    device claim error bound, extend the selection to every group whose
    claim could still reach top-k, and take the exact top-k. Exactness
    never depends on device numerics (groups cover ALL candidates).
"""
import numpy as np
import ml_dtypes

B = 256          # queries
N = 500000       # candidates
D = 128          # dim
NCORES = 8
NSH = N // NCORES          # 62500 real candidates per core
CHUNK = 2048               # candidates per chunk (4 PSUM banks f32)
NCHUNK = 31                # chunks per core
NSHP = NCHUNK * CHUNK      # 63488 padded candidates per core
FOLD = 2                   # candidates per claimed slot
SLOTS = CHUNK // FOLD      # 1024 slots per chunk
NSLOT = NCHUNK * SLOTS     # 31744 slots per (core, query)
PAIR_STRIDE = CHUNK // 2   # member j partner offset within chunk

_CACHE = {}


def build(loops=1, pattern="C", cbufs=4, ebufs=3, plbufs=2, prbufs=2):
    """Build + compile the per-core Bass program. Returns the compiled Bacc.

    Split-tile design: each 2048-chunk unit uses TWO 2-bank PSUM tiles —
    PR (cols 1024:2048, matmul'd first) and PL (cols 0:1024). Each PSUM
    tile has exactly ONE reader so tile recycling never waits on the
    chained ACT->DVE pair:
      C: ACT evacuates PR -> sc (overlapping PL's matmuls); DVE pair-max
         of PL (PSUM f32) x sc (SBUF bf16) at 1x.
      A: ACT evacuates PR and PL (two ops); DVE bf16 pair-max at 2x.
      V: DVE reduce_max over strided [128,1024,2] view spanning... not
         supported in split layout; kept only as diag on PL pairs.
      N: no drain (diag).
    All flows produce slot j = max(s_j, s_{j+1024}).
    """
    import concourse.bass as bass
    import concourse.tile as tile
    from concourse import bacc, mybir

    bf16 = mybir.dt.bfloat16
    f32 = mybir.dt.float32
    Copy = mybir.ActivationFunctionType.Copy

    nc = bacc.Bacc("TRN2", debug=False)
    qt = nc.dram_tensor("qt", [D, B], bf16, kind="ExternalInput").ap()
    ct = nc.dram_tensor("ct", [NCHUNK, D, CHUNK], bf16, kind="ExternalInput").ap()
    v8 = nc.dram_tensor("v8", [B, NSLOT], bf16, kind="ExternalOutput").ap()

    with tile.TileContext(nc) as tc:
        with (
            tc.tile_pool(name="qpool", bufs=1) as qpool,
            tc.tile_pool(name="cpool", bufs=cbufs) as cpool,
            tc.tile_pool(name="ppl", bufs=plbufs, space="PSUM") as ppl,
            tc.tile_pool(name="ppr", bufs=prbufs, space="PSUM") as ppr,
            tc.tile_pool(name="evac", bufs=ebufs) as epool,
            tc.tile_pool(name="acc", bufs=1) as accp,
        ):
            qtile = qpool.tile([D, B], bf16)
            nc.sync.dma_start(qtile[:], qt[:])
            vacc = [
                accp.tile([128, NSLOT], bf16, tag=f"vacc{h}", name=f"vacc{h}")
                for h in range(2)
            ]
            dummy = None
            if "D" in pattern:
                dummy = accp.tile([128, SLOTS], bf16, tag="dum", name="dum")
                nc.scalar.memzero(dummy[:])

            def body(_iv=None):
                u = 0
                pending = []

                def flush():
                    while pending:
                        pending.pop(0)()

                for cc in range(NCHUNK):
                    ctile = cpool.tile([D, CHUNK], bf16, tag="ct", name="ctile")
                    nc.sync.dma_start(ctile[:], ct[cc, :, :])
                    for h in range(2):
                        flow = pattern[u % len(pattern)]
                        dst = vacc[h][:, bass.ds(cc * SLOTS, SLOTS)]
                        lhsT = qtile[:, bass.ds(h * 128, 128)]
                        pr = ppr.tile([128, SLOTS], f32, tag="pr", name="pr")
                        pl = ppl.tile([128, SLOTS], f32, tag="pl", name="pl")
                        if flow != "N":
                            for j in range(2):
                                nc.tensor.matmul(
                                    pr[:, bass.ds(j * 512, 512)],
                                    lhsT=lhsT,
                                    rhs=ctile[:, bass.ds(SLOTS + j * 512, 512)],
                                    start=True,
                                    stop=True,
                                )
                        for j in range(2):
                            nc.tensor.matmul(
                                pl[:, bass.ds(j * 512, 512)],
                                lhsT=lhsT,
                                rhs=ctile[:, bass.ds(j * 512, 512)],
                                start=True,
                                stop=True,
                            )
                        if flow == "A":
                            # lazy emission: interleave this unit's ACT burst
                            # between neighbouring C-units' evacs so DVE's sc
                            # feed is never stalled behind a 2-op ACT run.
                            def emit_a(pr=pr, pl=pl, dst=dst):
                                sc = epool.tile(
                                    [128, CHUNK], bf16, tag="scA", name="scA"
                                )
                                nc.scalar.activation(
                                    sc[:, bass.ds(SLOTS, SLOTS)], pr[:], Copy
                                )
                                nc.scalar.activation(
                                    sc[:, bass.ds(0, SLOTS)], pl[:], Copy
                                )
                                nc.vector.tensor_max(
                                    dst,
                                    sc[:, bass.ds(0, SLOTS)],
                                    sc[:, bass.ds(SLOTS, SLOTS)],
                                )

                            pending.append(emit_a)
                        elif flow == "G":
                            # ACT evacuates both halves (lazily interleaved);
                            # GPSIMD does the pair-max — DVE untouched.
                            def emit_g(pr=pr, pl=pl, dst=dst):
                                sc = epool.tile(
                                    [128, CHUNK], f32, tag="scG", name="scG"
                                )
                                nc.scalar.activation(
                                    sc[:, bass.ds(SLOTS, SLOTS)], pr[:], Copy
                                )
                                nc.scalar.activation(
                                    sc[:, bass.ds(0, SLOTS)], pl[:], Copy
                                )
                                nc.gpsimd.tensor_max(
                                    dst,
                                    sc[:, bass.ds(0, SLOTS)],
                                    sc[:, bass.ds(SLOTS, SLOTS)],
                                )

                            pending.append(emit_g)
                        elif flow == "C":
                            sc = epool.tile([128, SLOTS], bf16, tag="scC", name="scC")
                            nc.scalar.activation(sc[:], pr[:], Copy)
                            nc.vector.tensor_max(dst, pl[:], sc[:])
                            flush()
                        elif flow == "D":  # diag: DVE mixed vs dummy (no ACT dep)
                            nc.vector.tensor_max(dst, pl[:], dummy[:])
                            # tiny ACT consume so pr isn't written-never-read
                            scm = epool.tile([128, 16], bf16, tag="scm", name="scm")
                            nc.scalar.activation(scm[:], pr[:, bass.ds(0, 16)], Copy)
                        elif flow == "S":  # diag: ACT evac only, tiny DVE
                            sc = epool.tile([128, SLOTS], bf16, tag="scC", name="scC")
                            sc2 = epool.tile([128, SLOTS], bf16, tag="sc2", name="sc2")
                            nc.scalar.activation(sc[:], pr[:], Copy)
                            nc.scalar.activation(sc2[:], pl[:], Copy)
                            nc.vector.tensor_max(
                                dst[:, bass.ds(0, 16)],
                                sc[:, bass.ds(0, 16)],
                                sc2[:, bass.ds(0, 16)],
                            )
                        u += 1
                flush()

            if loops == 1:
                body()
            else:
                with tc.For_i(0, loops, 1) as iv:
                    body(iv)

            for h in range(2):
                nc.sync.dma_start(v8[bass.ds(h * 128, 128), :], vacc[h][:])
    nc.compile()
    return nc


def _get_nc():
    if "nc" not in _CACHE:
        _CACHE["nc"] = build()
    return _CACHE["nc"]


def make_in_maps(queries, candidates):
    qt = np.ascontiguousarray(queries.T).astype(ml_dtypes.bfloat16)
    cb = candidates.astype(ml_dtypes.bfloat16)
    in_maps = []
    for c in range(NCORES):
        flat = np.zeros((D, NSHP), dtype=ml_dtypes.bfloat16)
        flat[:, :NSH] = cb[c * NSH : (c + 1) * NSH].T
        ct = np.ascontiguousarray(
            flat.reshape(D, NCHUNK, CHUNK).transpose(1, 0, 2)
        )
        in_maps.append({"qt": qt, "ct": ct})
    return in_maps


def _device_claims(queries, candidates):
    """Run the 8-core SPMD kernel; return claims [NCORES, B, NSLOT] f32."""
    from concourse.bass_utils import run_bass_kernel_spmd

    nc = _get_nc()
    in_maps = make_in_maps(queries, candidates)
    res = None
    for attempt in range(3):
        try:
            res = run_bass_kernel_spmd(nc, in_maps, core_ids=list(range(NCORES))).results
            break
        except Exception:
            if attempt == 2:
                raise
            import time as _time

            _time.sleep(2.0)
    assert res is not None
    return np.stack([r["v8"] for r in res]).astype(np.float32)


def kernel(queries, candidates, identifiers, k):
    queries = np.asarray(queries, dtype=np.float32)
    candidates = np.asarray(candidates, dtype=np.float32)
    identifiers = np.asarray(identifiers)
    kk = int(k)

    v8 = _device_claims(queries, candidates)            # [8, B, NSLOT]

    # flatten claims to [B, NCORES*NSLOT]; group g = (core, slotcol)
    vals = v8.transpose(1, 0, 2).reshape(B, NCORES * NSLOT)

    q64 = queries.astype(np.float64)
    sigma = np.linalg.norm(queries, axis=1)

    # group id -> member global candidate indices [..., FOLD] (or <0 invalid)
    def members_of(g):
        core, sl = g // NSLOT, g % NSLOT
        c, j = sl // SLOTS, sl % SLOTS
        L = (c * CHUNK + j)[..., None] + PAIR_STRIDE * np.arange(FOLD)
        valid = L < NSH
        gl = L + (core * NSH)[..., None]
        return np.where(valid, gl, -1)

    def rescore(mem, qidx):
        """mem [Q, M, FOLD] global ids (-1 invalid) -> exact f64 scores."""
        Q = mem.shape[0]
        out = np.empty(mem.shape, np.float64)
        step = 64
        for s in range(0, Q, step):
            e = min(s + step, Q)
            blk = mem[s:e]
            safe = np.where(blk >= 0, blk, 0)
            sv = np.einsum(
                "qmfd,qd->qmf",
                candidates[safe].astype(np.float64),
                q64[qidx[s:e]],
            )
            out[s:e] = np.where(blk >= 0, sv, -np.inf)
        return out

    # --- preselect top-C groups per query, rescore exactly ---
    C = max(2 * kk, kk + 64)
    part = np.argpartition(-vals, C, axis=1)[:, :C]
    vsel = np.take_along_axis(vals, part, 1)
    mem = members_of(part)                              # [B, C, FOLD]
    allq = np.arange(B)
    se = rescore(mem, allq)                             # [B, C, FOLD]
    gmax = se.max(2)
    finite = np.isfinite(gmax)
    delta = np.where(finite, np.abs(vsel - gmax), 0.0).max(1)
    margin = 4.0 * delta + 1e-3 * sigma

    flat = se.reshape(B, -1)
    vk = -np.partition(-flat, kk - 1, axis=1)[:, kk - 1]
    thr = vk - margin

    pool_v = [flat[q] for q in range(B)]
    pool_g = [mem[q].reshape(-1) for q in range(B)]

    # any group above thr that wasn't rescored yet
    selmask = np.zeros(vals.shape, dtype=bool)
    np.put_along_axis(selmask, part, True, 1)
    need = (vals >= thr[:, None]) & ~selmask
    for q in np.nonzero(need.any(1))[0]:
        g = np.nonzero(need[q])[0]
        m = members_of(g)[None]                          # [1, M, FOLD]
        sv = rescore(m, np.array([q]))[0]
        pool_v[q] = np.concatenate([pool_v[q], sv.reshape(-1)])
        pool_g[q] = np.concatenate([pool_g[q], m[0].reshape(-1)])

    # --- final exact top-k per query (dedupe, desc value, index tiebreak) --
    out_v = np.empty((B, kk), np.float32)
    out_g = np.empty((B, kk), np.int64)
    for q in range(B):
        keep = pool_g[q] >= 0
        g, first = np.unique(pool_g[q][keep], return_index=True)
        v32 = pool_v[q][keep][first].astype(np.float32)
        assert v32.size >= kk
        order = np.lexsort((g, -v32))[:kk]
        out_v[q] = v32[order]
        out_g[q] = g[order]

    top_ids = identifiers[out_g]
    return out_v, top_ids


# revision 14
# speedup vs baseline: 1.6602x; 1.0011x over previous
"""Distributed brute-force KNN (retrieval) kernel for one TRN2 chip (8 NeuronCores).

Problem: queries [256,128] f32, candidates [500000,128] f32, identifiers [500000] i32,
k=100. Output: (values [256,100] f32 desc-sorted, ids [256,100] i32).

Strategy (v3 — FOLD=2 pair-max claims, drain-balanced flows):
  - Shard candidates over N across the 8 cores (62500 each, zero-padded to
    63488 = 31 chunks x 2048).
  - Per core, per chunk (2048 cands) and query-half (128 queries): 4 bf16
    matmuls (Q stationary) -> one PSUM f32 tile [128, 2048] (4 banks).
  - Drain+fold to pair-max claims: slot j of a chunk claims
    max(s[j], s[j+1024]) in bf16 (members {base+j, base+j+1024}).
    Two pipelined flows balance ScalarE vs VectorE:
      A: ACT copies all 2048 f32 PSUM -> SBUF bf16 (1x); DVE folds with a
         bf16 tensor-max at 2x.
      C: ACT copies cols [1024:2048] only; DVE tensor-max mixes the PSUM
         f32 left half with the evacuated bf16 right half (1x).
      V: DVE reduce_max over a [128,1024,2]-strided PSUM view (1x,
         drains+folds in one op, zero ACT).
    Default pattern 8C:1A per the engine-balance LP (ACT ~ DVE ~ 70us).
  - Host: rescore the top-C claimed groups exactly in f64, derive the
    device claim error bound, extend the selection to every group whose
    claim could still reach top-k, and take the exact top-k. Exactness
    never depends on device numerics (groups cover ALL candidates).
"""
import numpy as np
import ml_dtypes

B = 256          # queries
N = 500000       # candidates
D = 128          # dim
NCORES = 8
NSH = N // NCORES          # 62500 real candidates per core
CHUNK = 2048               # candidates per chunk (4 PSUM banks f32)
NCHUNK = 31                # chunks per core
NSHP = NCHUNK * CHUNK      # 63488 padded candidates per core
FOLD = 2                   # candidates per claimed slot
SLOTS = CHUNK // FOLD      # 1024 slots per chunk
NSLOT = NCHUNK * SLOTS     # 31744 slots per (core, query)
PAIR_STRIDE = CHUNK // 2   # member j partner offset within chunk

_CACHE = {}


def build(loops=1, pattern="C", cbufs=4, ebufs=3, plbufs=2, prbufs=2):
    """Build + compile the per-core Bass program. Returns the compiled Bacc.

    Split-tile design: each 2048-chunk unit uses TWO 2-bank PSUM tiles —
    PR (cols 1024:2048, matmul'd first) and PL (cols 0:1024). Each PSUM
    tile has exactly ONE reader so tile recycling never waits on the
    chained ACT->DVE pair:
      C: ACT evacuates PR -> sc (overlapping PL's matmuls); DVE pair-max
         of PL (PSUM f32) x sc (SBUF bf16) at 1x.  [default, fastest]
      A: ACT evacuates PR and PL (two ops, lazily interleaved); DVE bf16
         pair-max at 2x.
      G: like A but GPSIMD does the fold (broken in this walrus build).
      D/S: timing diagnostics only (wrong claims for those units).
    All flows produce slot j = max(s_j, s_{j+1024}).
    """
    import concourse.bass as bass
    import concourse.tile as tile
    from concourse import bacc, mybir

    bf16 = mybir.dt.bfloat16
    f32 = mybir.dt.float32
    Copy = mybir.ActivationFunctionType.Copy

    nc = bacc.Bacc("TRN2", debug=False)
    qt = nc.dram_tensor("qt", [D, B], bf16, kind="ExternalInput").ap()
    ct = nc.dram_tensor("ct", [NCHUNK, D, CHUNK], bf16, kind="ExternalInput").ap()
    v8 = nc.dram_tensor("v8", [B, NSLOT], bf16, kind="ExternalOutput").ap()

    with tile.TileContext(nc) as tc:
        with (
            tc.tile_pool(name="qpool", bufs=1) as qpool,
            tc.tile_pool(name="cpool", bufs=cbufs) as cpool,
            tc.tile_pool(name="ppl", bufs=plbufs, space="PSUM") as ppl,
            tc.tile_pool(name="ppr", bufs=prbufs, space="PSUM") as ppr,
            tc.tile_pool(name="evac", bufs=ebufs) as epool,
            tc.tile_pool(name="acc", bufs=1) as accp,
        ):
            qtile = qpool.tile([D, B], bf16)
            nc.sync.dma_start(qtile[:], qt[:])
            vacc = [
                accp.tile([128, NSLOT], bf16, tag=f"vacc{h}", name=f"vacc{h}")
                for h in range(2)
            ]
            dummy = None
            if "D" in pattern:
                dummy = accp.tile([128, SLOTS], bf16, tag="dum", name="dum")
                nc.scalar.memzero(dummy[:])

            def body(_iv=None):
                u = 0
                pending = []

                def flush():
                    while pending:
                        pending.pop(0)()

                for cc in range(NCHUNK):
                    ctile = cpool.tile([D, CHUNK], bf16, tag="ct", name="ctile")
                    nc.sync.dma_start(ctile[:], ct[cc, :, :])
                    for h in range(2):
                        flow = pattern[u % len(pattern)]
                        dst = vacc[h][:, bass.ds(cc * SLOTS, SLOTS)]
                        lhsT = qtile[:, bass.ds(h * 128, 128)]
                        pr = ppr.tile([128, SLOTS], f32, tag="pr", name="pr")
                        pl = ppl.tile([128, SLOTS], f32, tag="pl", name="pl")
                        if flow != "N":
                            for j in range(2):
                                nc.tensor.matmul(
                                    pr[:, bass.ds(j * 512, 512)],
                                    lhsT=lhsT,
                                    rhs=ctile[:, bass.ds(SLOTS + j * 512, 512)],
                                    start=True,
                                    stop=True,
                                )
                        for j in range(2):
                            nc.tensor.matmul(
                                pl[:, bass.ds(j * 512, 512)],
                                lhsT=lhsT,
                                rhs=ctile[:, bass.ds(j * 512, 512)],
                                start=True,
                                stop=True,
                            )
                        if flow == "A":
                            # lazy emission: interleave this unit's ACT burst
                            # between neighbouring C-units' evacs so DVE's sc
                            # feed is never stalled behind a 2-op ACT run.
                            def emit_a(pr=pr, pl=pl, dst=dst):
                                sc = epool.tile(
                                    [128, CHUNK], bf16, tag="scA", name="scA"
                                )
                                nc.scalar.activation(
                                    sc[:, bass.ds(SLOTS, SLOTS)], pr[:], Copy
                                )
                                nc.scalar.activation(
                                    sc[:, bass.ds(0, SLOTS)], pl[:], Copy
                                )
                                nc.vector.tensor_max(
                                    dst,
                                    sc[:, bass.ds(0, SLOTS)],
                                    sc[:, bass.ds(SLOTS, SLOTS)],
                                )

                            pending.append(emit_a)
                        elif flow == "G":
                            # ACT evacuates both halves (lazily interleaved);
                            # GPSIMD does the pair-max — DVE untouched.
                            def emit_g(pr=pr, pl=pl, dst=dst):
                                sc = epool.tile(
                                    [128, CHUNK], f32, tag="scG", name="scG"
                                )
                                nc.scalar.activation(
                                    sc[:, bass.ds(SLOTS, SLOTS)], pr[:], Copy
                                )
                                nc.scalar.activation(
                                    sc[:, bass.ds(0, SLOTS)], pl[:], Copy
                                )
                                nc.gpsimd.tensor_max(
                                    dst,
                                    sc[:, bass.ds(0, SLOTS)],
                                    sc[:, bass.ds(SLOTS, SLOTS)],
                                )

                            pending.append(emit_g)
                        elif flow == "C":
                            sc = epool.tile([128, SLOTS], bf16, tag="scC", name="scC")
                            nc.scalar.activation(sc[:], pr[:], Copy)
                            nc.vector.tensor_max(dst, pl[:], sc[:])
                            flush()
                        elif flow == "D":  # diag: DVE mixed vs dummy (no ACT dep)
                            nc.vector.tensor_max(dst, pl[:], dummy[:])
                            # tiny ACT consume so pr isn't written-never-read
                            scm = epool.tile([128, 16], bf16, tag="scm", name="scm")
                            nc.scalar.activation(scm[:], pr[:, bass.ds(0, 16)], Copy)
                        elif flow == "S":  # diag: ACT evac only, tiny DVE
                            sc = epool.tile([128, SLOTS], bf16, tag="scC", name="scC")
                            sc2 = epool.tile([128, SLOTS], bf16, tag="sc2", name="sc2")
                            nc.scalar.activation(sc[:], pr[:], Copy)
                            nc.scalar.activation(sc2[:], pl[:], Copy)
                            nc.vector.tensor_max(
                                dst[:, bass.ds(0, 16)],
                                sc[:, bass.ds(0, 16)],
                                sc2[:, bass.ds(0, 16)],
                            )
                        u += 1
                flush()

            if loops == 1:
                body()
            else:
                with tc.For_i(0, loops, 1) as iv:
                    body(iv)

            for h in range(2):
                nc.sync.dma_start(v8[bass.ds(h * 128, 128), :], vacc[h][:])
    nc.compile()
    return nc


def _get_nc():
    if "nc" not in _CACHE:
        _CACHE["nc"] = build()
    return _CACHE["nc"]


def make_in_maps(queries, candidates):
    qt = np.ascontiguousarray(queries.T).astype(ml_dtypes.bfloat16)
    cb = candidates.astype(ml_dtypes.bfloat16)
    in_maps = []
    for c in range(NCORES):
        flat = np.zeros((D, NSHP), dtype=ml_dtypes.bfloat16)
        flat[:, :NSH] = cb[c * NSH : (c + 1) * NSH].T
        ct = np.ascontiguousarray(
            flat.reshape(D, NCHUNK, CHUNK).transpose(1, 0, 2)
        )
        in_maps.append({"qt": qt, "ct": ct})
    return in_maps


def _device_claims(queries, candidates):
    """Run the 8-core SPMD kernel; return claims [NCORES, B, NSLOT] f32."""
    from concourse.bass_utils import run_bass_kernel_spmd

    nc = _get_nc()
    in_maps = make_in_maps(queries, candidates)
    res = None
    for attempt in range(3):
        try:
            res = run_bass_kernel_spmd(nc, in_maps, core_ids=list(range(NCORES))).results
            break
        except Exception:
            if attempt == 2:
                raise
            import time as _time

            _time.sleep(2.0)
    assert res is not None
    return np.stack([r["v8"] for r in res]).astype(np.float32)


def kernel(queries, candidates, identifiers, k):
    queries = np.asarray(queries, dtype=np.float32)
    candidates = np.asarray(candidates, dtype=np.float32)
    identifiers = np.asarray(identifiers)
    kk = int(k)

    v8 = _device_claims(queries, candidates)            # [8, B, NSLOT]

    # flatten claims to [B, NCORES*NSLOT]; group g = (core, slotcol)
    vals = v8.transpose(1, 0, 2).reshape(B, NCORES * NSLOT)

    q64 = queries.astype(np.float64)
    sigma = np.linalg.norm(queries, axis=1)

    # group id -> member global candidate indices [..., FOLD] (or <0 invalid)
    def members_of(g):
        core, sl = g // NSLOT, g % NSLOT
        c, j = sl // SLOTS, sl % SLOTS
        L = (c * CHUNK + j)[..., None] + PAIR_STRIDE * np.arange(FOLD)
        valid = L < NSH
        gl = L + (core * NSH)[..., None]
        return np.where(valid, gl, -1)

    def rescore(mem, qidx):
        """mem [Q, M, FOLD] global ids (-1 invalid) -> exact f64 scores."""
        Q = mem.shape[0]
        out = np.empty(mem.shape, np.float64)
        step = 64
        for s in range(0, Q, step):
            e = min(s + step, Q)
            blk = mem[s:e]
            safe = np.where(blk >= 0, blk, 0)
            sv = np.einsum(
                "qmfd,qd->qmf",
                candidates[safe].astype(np.float64),
                q64[qidx[s:e]],
            )
            out[s:e] = np.where(blk >= 0, sv, -np.inf)
        return out

    # --- preselect top-C groups per query, rescore exactly ---
    C = max(2 * kk, kk + 64)
    part = np.argpartition(-vals, C, axis=1)[:, :C]
    vsel = np.take_along_axis(vals, part, 1)
    mem = members_of(part)                              # [B, C, FOLD]
    allq = np.arange(B)
    se = rescore(mem, allq)                             # [B, C, FOLD]
    gmax = se.max(2)
    finite = np.isfinite(gmax)
    delta = np.where(finite, np.abs(vsel - gmax), 0.0).max(1)
    margin = 4.0 * delta + 1e-3 * sigma

    flat = se.reshape(B, -1)
    vk = -np.partition(-flat, kk - 1, axis=1)[:, kk - 1]
    thr = vk - margin

    pool_v = [flat[q] for q in range(B)]
    pool_g = [mem[q].reshape(-1) for q in range(B)]

    # any group above thr that wasn't rescored yet
    selmask = np.zeros(vals.shape, dtype=bool)
    np.put_along_axis(selmask, part, True, 1)
    need = (vals >= thr[:, None]) & ~selmask
    for q in np.nonzero(need.any(1))[0]:
        g = np.nonzero(need[q])[0]
        m = members_of(g)[None]                          # [1, M, FOLD]
        sv = rescore(m, np.array([q]))[0]
        pool_v[q] = np.concatenate([pool_v[q], sv.reshape(-1)])
        pool_g[q] = np.concatenate([pool_g[q], m[0].reshape(-1)])

    # --- final exact top-k per query (dedupe, desc value, index tiebreak) --
    out_v = np.empty((B, kk), np.float32)
    out_g = np.empty((B, kk), np.int64)
    for q in range(B):
        keep = pool_g[q] >= 0
        g, first = np.unique(pool_g[q][keep], return_index=True)
        v32 = pool_v[q][keep][first].astype(np.float32)
        assert v32.size >= kk
        order = np.lexsort((g, -v32))[:kk]
        out_v[q] = v32[order]
        out_g[q] = g[order]

    top_ids = identifiers[out_g]
    return out_v, top_ids


# revision 19
# speedup vs baseline: 1.6692x; 1.0054x over previous
"""Distributed brute-force KNN (retrieval) kernel for one TRN2 chip (8 NeuronCores).

Problem: queries [256,128] f32, candidates [500000,128] f32, identifiers [500000] i32,
k=100. Output: (values [256,100] f32 desc-sorted, ids [256,100] i32).

Strategy (v3 — FOLD=2 pair-max claims, drain-balanced flows):
  - Shard candidates over N across the 8 cores (62500 each, zero-padded to
    63488 = 31 chunks x 2048).
  - Per core, per chunk (2048 cands) and query-half (128 queries): 4 bf16
    matmuls (Q stationary) -> one PSUM f32 tile [128, 2048] (4 banks).
  - Drain+fold to pair-max claims: slot j of a chunk claims
    max(s[j], s[j+1024]) in bf16 (members {base+j, base+j+1024}).
    Two pipelined flows balance ScalarE vs VectorE:
      A: ACT copies all 2048 f32 PSUM -> SBUF bf16 (1x); DVE folds with a
         bf16 tensor-max at 2x.
      C: ACT copies cols [1024:2048] only; DVE tensor-max mixes the PSUM
         f32 left half with the evacuated bf16 right half (1x).
      V: DVE reduce_max over a [128,1024,2]-strided PSUM view (1x,
         drains+folds in one op, zero ACT).
    Default pattern 8C:1A per the engine-balance LP (ACT ~ DVE ~ 70us).
  - Host: rescore the top-C claimed groups exactly in f64, derive the
    device claim error bound, extend the selection to every group whose
    claim could still reach top-k, and take the exact top-k. Exactness
    never depends on device numerics (groups cover ALL candidates).
"""
import numpy as np
import ml_dtypes

B = 256          # queries
N = 500000       # candidates
D = 128          # dim
NCORES = 8
NSH = N // NCORES          # 62500 real candidates per core
CHUNK = 2048               # candidates per chunk (4 PSUM banks f32)
NCHUNK = 31                # chunks per core
NSHP = NCHUNK * CHUNK      # 63488 padded candidates per core
FOLD = 2                   # candidates per claimed slot
SLOTS = CHUNK // FOLD      # 1024 slots per chunk
NSLOT = NCHUNK * SLOTS     # 31744 slots per (core, query)
PAIR_STRIDE = CHUNK // 2   # member j partner offset within chunk

_CACHE = {}


def build(loops=1, pattern="C", cbufs=4, ebufs=3, plbufs=2, prbufs=2, dmapair=False,
          unroll=1, staggered=False):
    """Build + compile the per-core Bass program. Returns the compiled Bacc.

    Split-tile design: each 2048-chunk unit uses TWO 2-bank PSUM tiles —
    PR (cols 1024:2048, matmul'd first) and PL (cols 0:1024). Each PSUM
    tile has exactly ONE reader so tile recycling never waits on the
    chained ACT->DVE pair:
      C: ACT evacuates PR -> sc (overlapping PL's matmuls); DVE pair-max
         of PL (PSUM f32) x sc (SBUF bf16) at 1x.  [default, fastest]
      A: ACT evacuates PR and PL (two ops, lazily interleaved); DVE bf16
         pair-max at 2x.
      G: like A but GPSIMD does the fold (broken in this walrus build).
      D/S: timing diagnostics only (wrong claims for those units).
    All flows produce slot j = max(s_j, s_{j+1024}).
    """
    import concourse.bass as bass
    import concourse.tile as tile
    from concourse import bacc, mybir

    bf16 = mybir.dt.bfloat16
    f32 = mybir.dt.float32
    Copy = mybir.ActivationFunctionType.Copy

    nc = bacc.Bacc("TRN2", debug=False)
    qt = nc.dram_tensor("qt", [D, B], bf16, kind="ExternalInput").ap()
    ct = nc.dram_tensor("ct", [NCHUNK, D, CHUNK], bf16, kind="ExternalInput").ap()
    v8 = nc.dram_tensor("v8", [B, NSLOT], bf16, kind="ExternalOutput").ap()

    with tile.TileContext(nc) as tc:
        with (
            tc.tile_pool(name="qpool", bufs=1) as qpool,
            tc.tile_pool(name="cpool", bufs=cbufs) as cpool,
            tc.tile_pool(name="ppl", bufs=plbufs, space="PSUM") as ppl,
            tc.tile_pool(name="ppr", bufs=prbufs, space="PSUM") as ppr,
            tc.tile_pool(name="evac", bufs=ebufs) as epool,
            tc.tile_pool(name="acc", bufs=1) as accp,
        ):
            qtile = qpool.tile([D, B], bf16)
            nc.sync.dma_start(qtile[:], qt[:])
            vacc = [
                accp.tile([128, NSLOT], bf16, tag=f"vacc{h}", name=f"vacc{h}")
                for h in range(2)
            ]
            dummy = None
            if "D" in pattern:
                dummy = accp.tile([128, SLOTS], bf16, tag="dum", name="dum")
                nc.scalar.memzero(dummy[:])

            def body(_iv=None):
                u = 0
                pending = []
                ctile2 = None

                def flush():
                    while pending:
                        pending.pop(0)()

                for cc in range(NCHUNK):
                    if dmapair:
                        # one DMA covers two chunks (halves dma_start count;
                        # SWDGE descriptor work contends with DVE's SBUF port)
                        if cc % 2 == 0:
                            ctile2 = cpool.tile(
                                [D, 2 * CHUNK], bf16, tag="ct2", name="ctile2"
                            )
                            n = min(2, NCHUNK - cc)
                            src = ct[bass.ds(cc, n), :, :].rearrange(
                                "n d k -> d n k"
                            )
                            dst2 = ctile2[:, bass.ds(0, n * CHUNK)].rearrange(
                                "d (n k) -> d n k", n=n
                            )
                            nc.sync.dma_start(dst2, src)
                        ctile = ctile2[:, bass.ds((cc % 2) * CHUNK, CHUNK)]
                    else:
                        ctile = cpool.tile([D, CHUNK], bf16, tag="ct", name="ctile")
                        nc.sync.dma_start(ctile[:], ct[cc, :, :])
                    for h in range(2):
                        flow = pattern[u % len(pattern)]
                        dst = vacc[h][:, bass.ds(cc * SLOTS, SLOTS)]
                        lhsT = qtile[:, bass.ds(h * 128, 128)]
                        pr = ppr.tile([128, SLOTS], f32, tag="pr", name="pr")
                        pl = ppl.tile([128, SLOTS], f32, tag="pl", name="pl")
                        if flow != "N":
                            for j in range(2):
                                nc.tensor.matmul(
                                    pr[:, bass.ds(j * 512, 512)],
                                    lhsT=lhsT,
                                    rhs=ctile[:, bass.ds(SLOTS + j * 512, 512)],
                                    start=True,
                                    stop=True,
                                )
                        for j in range(2):
                            nc.tensor.matmul(
                                pl[:, bass.ds(j * 512, 512)],
                                lhsT=lhsT,
                                rhs=ctile[:, bass.ds(j * 512, 512)],
                                start=True,
                                stop=True,
                            )
                        if flow == "A":
                            # lazy emission: interleave this unit's ACT burst
                            # between neighbouring C-units' evacs so DVE's sc
                            # feed is never stalled behind a 2-op ACT run.
                            def emit_a(pr=pr, pl=pl, dst=dst):
                                sc = epool.tile(
                                    [128, CHUNK], bf16, tag="scA", name="scA"
                                )
                                nc.scalar.activation(
                                    sc[:, bass.ds(SLOTS, SLOTS)], pr[:], Copy
                                )
                                nc.scalar.activation(
                                    sc[:, bass.ds(0, SLOTS)], pl[:], Copy
                                )
                                nc.vector.tensor_max(
                                    dst,
                                    sc[:, bass.ds(0, SLOTS)],
                                    sc[:, bass.ds(SLOTS, SLOTS)],
                                )

                            pending.append(emit_a)
                        elif flow == "G":
                            # ACT evacuates both halves (lazily interleaved);
                            # GPSIMD does the pair-max — DVE untouched.
                            def emit_g(pr=pr, pl=pl, dst=dst):
                                sc = epool.tile(
                                    [128, CHUNK], f32, tag="scG", name="scG"
                                )
                                nc.scalar.activation(
                                    sc[:, bass.ds(SLOTS, SLOTS)], pr[:], Copy
                                )
                                nc.scalar.activation(
                                    sc[:, bass.ds(0, SLOTS)], pl[:], Copy
                                )
                                nc.gpsimd.tensor_max(
                                    dst,
                                    sc[:, bass.ds(0, SLOTS)],
                                    sc[:, bass.ds(SLOTS, SLOTS)],
                                )

                            pending.append(emit_g)
                        elif flow == "C":
                            sc = epool.tile([128, SLOTS], bf16, tag="scC", name="scC")
                            nc.scalar.activation(sc[:], pr[:], Copy)
                            nc.vector.tensor_max(dst, pl[:], sc[:])
                            flush()
                        elif flow == "D":  # diag: DVE mixed vs dummy (no ACT dep)
                            nc.vector.tensor_max(dst, pl[:], dummy[:])
                            # tiny ACT consume so pr isn't written-never-read
                            scm = epool.tile([128, 16], bf16, tag="scm", name="scm")
                            nc.scalar.activation(scm[:], pr[:, bass.ds(0, 16)], Copy)
                        elif flow == "S":  # diag: ACT evac only, tiny DVE
                            sc = epool.tile([128, SLOTS], bf16, tag="scC", name="scC")
                            sc2 = epool.tile([128, SLOTS], bf16, tag="sc2", name="sc2")
                            nc.scalar.activation(sc[:], pr[:], Copy)
                            nc.scalar.activation(sc2[:], pl[:], Copy)
                            nc.vector.tensor_max(
                                dst[:, bass.ds(0, 16)],
                                sc[:, bass.ds(0, 16)],
                                sc2[:, bass.ds(0, 16)],
                            )
                        u += 1
                flush()

            if loops == 1:
                for _ in range(unroll):
                    body()
            else:
                with tc.For_i(0, loops, 1, staggered_reset=staggered) as iv:
                    for _ in range(unroll):
                        body(iv)

            for h in range(2):
                nc.sync.dma_start(v8[bass.ds(h * 128, 128), :], vacc[h][:])
    nc.compile()
    return nc


def _get_nc():
    if "nc" not in _CACHE:
        _CACHE["nc"] = build()
    return _CACHE["nc"]


def make_in_maps(queries, candidates):
    qt = np.ascontiguousarray(queries.T).astype(ml_dtypes.bfloat16)
    cb = candidates.astype(ml_dtypes.bfloat16)
    in_maps = []
    for c in range(NCORES):
        flat = np.zeros((D, NSHP), dtype=ml_dtypes.bfloat16)
        flat[:, :NSH] = cb[c * NSH : (c + 1) * NSH].T
        ct = np.ascontiguousarray(
            flat.reshape(D, NCHUNK, CHUNK).transpose(1, 0, 2)
        )
        in_maps.append({"qt": qt, "ct": ct})
    return in_maps


def _device_claims(queries, candidates):
    """Run the 8-core SPMD kernel; return claims [NCORES, B, NSLOT] f32."""
    from concourse.bass_utils import run_bass_kernel_spmd

    nc = _get_nc()
    in_maps = make_in_maps(queries, candidates)
    res = None
    for attempt in range(3):
        try:
            res = run_bass_kernel_spmd(nc, in_maps, core_ids=list(range(NCORES))).results
            break
        except Exception:
            if attempt == 2:
                raise
            import time as _time

            _time.sleep(2.0)
    assert res is not None
    return np.stack([r["v8"] for r in res]).astype(np.float32)


def kernel(queries, candidates, identifiers, k):
    queries = np.asarray(queries, dtype=np.float32)
    candidates = np.asarray(candidates, dtype=np.float32)
    identifiers = np.asarray(identifiers)
    kk = int(k)

    v8 = _device_claims(queries, candidates)            # [8, B, NSLOT]

    # flatten claims to [B, NCORES*NSLOT]; group g = (core, slotcol)
    vals = v8.transpose(1, 0, 2).reshape(B, NCORES * NSLOT)

    q64 = queries.astype(np.float64)
    sigma = np.linalg.norm(queries, axis=1)

    # group id -> member global candidate indices [..., FOLD] (or <0 invalid)
    def members_of(g):
        core, sl = g // NSLOT, g % NSLOT
        c, j = sl // SLOTS, sl % SLOTS
        L = (c * CHUNK + j)[..., None] + PAIR_STRIDE * np.arange(FOLD)
        valid = L < NSH
        gl = L + (core * NSH)[..., None]
        return np.where(valid, gl, -1)

    def rescore(mem, qidx):
        """mem [Q, M, FOLD] global ids (-1 invalid) -> exact f64 scores."""
        Q = mem.shape[0]
        out = np.empty(mem.shape, np.float64)
        step = 64
        for s in range(0, Q, step):
            e = min(s + step, Q)
            blk = mem[s:e]
            safe = np.where(blk >= 0, blk, 0)
            sv = np.einsum(
                "qmfd,qd->qmf",
                candidates[safe].astype(np.float64),
                q64[qidx[s:e]],
            )
            out[s:e] = np.where(blk >= 0, sv, -np.inf)
        return out

    # --- preselect top-C groups per query, rescore exactly ---
    C = max(2 * kk, kk + 64)
    part = np.argpartition(-vals, C, axis=1)[:, :C]
    vsel = np.take_along_axis(vals, part, 1)
    mem = members_of(part)                              # [B, C, FOLD]
    allq = np.arange(B)
    se = rescore(mem, allq)                             # [B, C, FOLD]
    gmax = se.max(2)
    finite = np.isfinite(gmax)
    delta = np.where(finite, np.abs(vsel - gmax), 0.0).max(1)
    margin = 4.0 * delta + 1e-3 * sigma

    flat = se.reshape(B, -1)
    vk = -np.partition(-flat, kk - 1, axis=1)[:, kk - 1]
    thr = vk - margin

    pool_v = [flat[q] for q in range(B)]
    pool_g = [mem[q].reshape(-1) for q in range(B)]

    # any group above thr that wasn't rescored yet
    selmask = np.zeros(vals.shape, dtype=bool)
    np.put_along_axis(selmask, part, True, 1)
    need = (vals >= thr[:, None]) & ~selmask
    for q in np.nonzero(need.any(1))[0]:
        g = np.nonzero(need[q])[0]
        m = members_of(g)[None]                          # [1, M, FOLD]
        sv = rescore(m, np.array([q]))[0]
        pool_v[q] = np.concatenate([pool_v[q], sv.reshape(-1)])
        pool_g[q] = np.concatenate([pool_g[q], m[0].reshape(-1)])

    # --- final exact top-k per query (dedupe, desc value, index tiebreak) --
    out_v = np.empty((B, kk), np.float32)
    out_g = np.empty((B, kk), np.int64)
    for q in range(B):
        keep = pool_g[q] >= 0
        g, first = np.unique(pool_g[q][keep], return_index=True)
        v32 = pool_v[q][keep][first].astype(np.float32)
        assert v32.size >= kk
        order = np.lexsort((g, -v32))[:kk]
        out_v[q] = v32[order]
        out_g[q] = g[order]

    top_ids = identifiers[out_g]
    return out_v, top_ids
